# revision 31
# baseline (speedup 1.0000x reference)
"""BitNetAttention Trainium2 kernel (nn_BitNetAttention, B=2 S=2048 HID=2560).

Reference: q/k/v projections (x @ W^T), RoPE (rotate-half, theta=5e5), causal
GQA attention (20 q heads, 5 kv heads, head_dim 128), BitNetSubNorm per-channel
gain, o_proj.

Sharding across 8 NeuronCores: core c handles batch c//4 and 5 query heads:
with g = c%4, q heads [4g..4g+3, 16+g], kv heads [g, 4]. This grouping makes
the local head->kv map the constant [0,0,0,0,1] so one SPMD program serves all
cores. Each core computes its batch's partial o_proj output (sum over its 5
heads); the host sums 4 partials per batch. sub_w is folded into wo on host.

Design (all-bf16 data, fp32 PSUM accumulation; measured ~285us/rep on HW,
rel err 3.4e-3 vs the fp32 reference):
  - x, wq, wk, wv, wo, cos/sin staged bf16; x resident in SBUF for the whole
    A phase (loaded once). DMA is issued piece-by-piece (weights interleaved
    with x hid-slices) so the K chains chase the loads with no PE idle.
  - A1-K: the 8 K accumulation chains (4 t-blocks x 2 kv heads) interleave at
    x-piece granularity across all 8 PSUM banks. RoPE: ACT drains PSUM via
    straight + rotate-half bf16 copies (sign folded into sinT), then DVE
    multiplies with the bf16 cos/sin tables and adds (all 2-byte SBUF ops).
  - A1-V/A2-Q: straight 20-link chains; V copies to [t,k-tile,d] bf16 on ACT.
  - BC: per (512-wide q-chunk, head): QK^T in sT layout [k,q] (bf16) with
    double-buffered score PSUM, exp on ACT (scale=1/sqrt(128), no max
    subtraction -- scores are O(1) gaussians; 1024-wide paired exp off the
    diagonal), causal triangular 0/1 mask on diagonal tiles, PV accumulating
    in PSUM over k-tiles. Softmax denominators: DVE pair-adds of exp tiles
    into an fp16 esum (fast 2-byte DVE mode; sums << fp16 max), one
    ones-matmul partition-reduce per (head, chunk), reciprocal+mul.
  - o_proj of chunk c-1 is emitted in 4-tile slices INSIDE chunk c's head
    loop: its PE work fills the PE while ACT works through that chunk's exps
    (the attention sections alone are exp-bound). y written bf16, one DMA
    per 128-row group; host sums the 4 partials per batch in fp32.
Tried and rejected on HW measurement: fp8 E4M3 DoubleRow projections (slower
than bf16 -- 329us vs 281us -- and rel err 3.2e-2 > the 2e-2 gate), QK
moving-side head pairing (walrus codegen rejects 3D strided moving APs),
deeper tile pools (app 4 / bep 4 / esp 3 / ycp 2 regressed to 372us).
"""

import numpy as np
import ml_dtypes
from contextlib import ExitStack

import concourse.bass as bass
import concourse.mybir as mybir
import concourse.tile as tile
from concourse.bass_utils import run_bass_kernel_spmd

F32 = mybir.dt.float32
F16 = mybir.dt.float16
BF16 = mybir.dt.bfloat16
FP8 = mybir.dt.float8e4
DR = mybir.MatmulPerfMode.DoubleRow

# fp8 (E4M3) DoubleRow projections. Measured on HW: SLOWER than bf16 (329us
# vs 281us) and rel err 3.2e-2 > 2e-2 gate. Keep off; code path retained.
FP8_PROJ = False
X_SCALE = 8.0
W_SCALE = 64.0

B, S, HID = 2, 2048, 2560
NH, NKV, HD = 20, 5, 128
G = NH // NKV
THETA = 500000.0
NCORES = 8
HEADS = 5          # query heads per core
KV = 2             # kv heads per core
KVIDX = [0, 0, 0, 0, 1]   # local head -> local kv head
HT = HID // 128    # 20 hidden k-tiles
BLK = 512          # t block width for projections
NBLK = S // BLK    # 4
CH = 512           # attention q-chunk width
NCH = S // CH      # 4
KT = S // 128      # 16 k-tiles
NO = HID // CH     # 5 o_proj hid chunks
SCALE = HD ** -0.5

_CACHE = {}


def _split_waits(nc):
    """Walrus ISA structs carry a single sync-wait slot. Move surplus waits
    onto EventSemaphore sequencer instructions inserted just before (same
    engine; engines are in-order so hoisting waits earlier is safe)."""
    import concourse.mybir as mb
    n_ev = 0
    for f in nc.m.functions:
        for bb in f.blocks:
            out = []
            changed = False
            for inst in bb.instructions:
                si = getattr(inst, "sync_info", None)
                if (type(inst).__name__ != "InstEventSemaphore" and si is not None
                        and len(si.on_wait) > 1):
                    waits = list(si.on_wait)
                    for w in waits[:-1]:
                        ev = mb.InstEventSemaphore(name=f"I-evw-{n_ev}", ins=[], outs=[])
                        n_ev += 1
                        ev.engine = inst.engine
                        ev.sync_info = mb.SyncInfo(on_wait=[w], on_update=[])
                        nc.register_instruction(ev)
                        out.append(ev)
                    inst.sync_info = mb.SyncInfo(on_wait=waits[-1:],
                                                 on_update=list(si.on_update))
                    changed = True
                out.append(inst)
            if changed:
                bb.instructions = out
    return n_ev


def build_nc(reps=1):
    nc = bass.Bass()
    xdt = FP8 if FP8_PROJ else BF16
    xT = nc.declare_dram_parameter("xT", [HID, S], xdt, isOutput=False)
    wqT = nc.declare_dram_parameter("wqT", [HID, HEADS * HD], xdt, isOutput=False)
    wkT = nc.declare_dram_parameter("wkT", [HID, KV * HD], xdt, isOutput=False)
    wvT = nc.declare_dram_parameter("wvT", [HID, KV * HD], xdt, isOutput=False)
    woT = nc.declare_dram_parameter("woT", [HEADS * HD, HID], BF16, isOutput=False)
    cosT = nc.declare_dram_parameter("cosT", [HD, S], BF16, isOutput=False)
    sinT = nc.declare_dram_parameter("sinT", [HD, S], BF16, isOutput=False)  # sign-folded
    mask = nc.declare_dram_parameter("mask", [128, 128], BF16, isOutput=False)  # triu 0/1
    ones = nc.declare_dram_parameter("ones", [128, 128], BF16, isOutput=False)
    y = nc.declare_dram_parameter("y", [S, HID], BF16, isOutput=True)

    if FP8_PROJ:
        # hid = 256*a + 128*j + p: 3D [p, 2, free] APs for DoubleRow matmuls
        xT_t = xT.rearrange("(a j p) t -> p a j t", p=128, j=2)    # [128, 10, 2, 2048]
        wqT_t = wqT.rearrange("(a j p) d -> p a j d", p=128, j=2)  # [128, 10, 2, 640]
        wkT_t = wkT.rearrange("(a j p) d -> p a j d", p=128, j=2)
        wvT_t = wvT.rearrange("(a j p) d -> p a j d", p=128, j=2)
    else:
        xT_t = xT.rearrange("(a p) t -> p a t", p=128)      # [128, 20, 2048]
        wqT_t = wqT.rearrange("(a p) d -> p a d", p=128)    # [128, 20, 640]
        wkT_t = wkT.rearrange("(a p) d -> p a d", p=128)
        wvT_t = wvT.rearrange("(a p) d -> p a d", p=128)
    woT_t = woT.rearrange("(h p) n -> p h n", p=128)    # [128, 5, 2560]
    y_r = y.rearrange("(c tt p) n -> p c tt n", p=128, tt=4)  # [128, 4, 4, 2560]

    with tile.TileContext(nc) as tc:
      for rep in range(reps):
        with ExitStack() as ctx:
            # ---------- persistent tiles (live through BC) ----------
            per = ctx.enter_context(tc.tile_pool(name=f"persist{rep}", bufs=1))
            kT_sb = per.tile([128, KV, S], BF16)        # 8KB/part
            v_sb = per.tile([128, KT, KV * HD], BF16)   # 8KB/part [t%128, ktile, kv*128+d]
            qT_sb = per.tile([128, HEADS, S], BF16)     # 20KB/part
            mask_sb = per.tile([128, 128], BF16)
            ones_sb = per.tile([128, 128], BF16)
            nc.sync.dma_start(out=mask_sb, in_=mask[:])
            nc.sync.dma_start(out=ones_sb, in_=ones[:])

            # ---------- phase A: projections (x resident) ----------
            with ExitStack() as actx:
                axp = actx.enter_context(tc.tile_pool(name=f"a_x{rep}", bufs=1))
                if FP8_PROJ:
                    x_sb = axp.tile([128, HT // 2, 2, S], FP8)  # 40KB/part
                else:
                    x_sb = axp.tile([128, HT, S], BF16)  # 80KB/part
                cos_sb = axp.tile([HD, S], BF16)         # 4KB
                sin_sb = axp.tile([HD, S], BF16)         # 4KB
                ropep = actx.enter_context(tc.tile_pool(name=f"a_rope{rep}", bufs=2))

                def rope(dst, psrc, t0, w):
                    # ACT drains the PSUM bank fast (straight + rotate-half
                    # copies; partition-offset reads legal from PSUM); DVE
                    # then works all-bf16 SBUF with aligned base partitions
                    pb = ropep.tile([128, BLK], BF16, tag="pb")
                    pbr = ropep.tile([128, BLK], BF16, tag="pbr")
                    nc.scalar.copy(pb[:, 0:w], psrc)
                    nc.scalar.copy(pbr[0:64, 0:w], psrc[64:128, :])
                    nc.scalar.copy(pbr[64:128, 0:w], psrc[0:64, :])
                    t1 = ropep.tile([128, BLK], BF16, tag="t1")
                    t2 = ropep.tile([128, BLK], BF16, tag="t2")
                    nc.vector.tensor_mul(t1[:, 0:w], pb[:, 0:w], cos_sb[:, t0:t0 + w])
                    nc.vector.tensor_mul(t2[:, 0:w], pbr[:, 0:w], sin_sb[:, t0:t0 + w])
                    nc.vector.tensor_add(dst, t1[:, 0:w], t2[:, 0:w])

                # ---- A1: K/V projections ----
                NA = HT // 2 if FP8_PROJ else HT   # contraction tiles
                pm = DR if FP8_PROJ else None
                awp = actx.enter_context(tc.tile_pool(name=f"a_w{rep}", bufs=1))
                awq = actx.enter_context(tc.tile_pool(name=f"q_w{rep}", bufs=1))
                if FP8_PROJ:
                    wk_sb = awp.tile([128, NA, 2, KV * HD], FP8)     # 5KB/part
                    wv_sb = awp.tile([128, NA, 2, KV * HD], FP8)     # 5KB/part
                    wq_sb = awq.tile([128, NA, 2, HEADS * HD], FP8)  # 12.5KB/part
                    pieces = [0, 1, 2, 4, 6, 8, NA]
                else:
                    wk_sb = awp.tile([128, NA, KV * HD], BF16)   # 10KB/part
                    wv_sb = awp.tile([128, NA, KV * HD], BF16)   # 10KB/part
                    wq_sb = awq.tile([128, NA, HEADS * HD], BF16)  # 25KB/part
                    pieces = [0, 2, 4, 8, 12, 16, NA]

                def wslice(w_sb, a, d0, d1):
                    return w_sb[:, a, :, d0:d1] if FP8_PROJ else w_sb[:, a, d0:d1]

                def xslice(a, t0, t1):
                    return x_sb[:, a, :, t0:t1] if FP8_PROJ else x_sb[:, a, t0:t1]

                # interleave weight and x loads piece-by-piece: the shared DMA
                # path delivers exactly what the K chains need next
                for pc in range(len(pieces) - 1):
                    a0, a1 = pieces[pc], pieces[pc + 1]
                    nc.sync.dma_start(out=wk_sb[:, a0:a1], in_=wkT_t[:, a0:a1])
                    nc.sync.dma_start(out=x_sb[:, a0:a1], in_=xT_t[:, a0:a1])
                    if pc == 1:
                        nc.sync.dma_start(out=cos_sb, in_=cosT[:])
                        nc.sync.dma_start(out=sin_sb, in_=sinT[:])
                for pc in range(len(pieces) - 1):
                    a0, a1 = pieces[pc], pieces[pc + 1]
                    nc.sync.dma_start(out=wv_sb[:, a0:a1], in_=wvT_t[:, a0:a1])
                for pc in range(len(pieces) - 1):
                    a0, a1 = pieces[pc], pieces[pc + 1]
                    nc.sync.dma_start(out=wq_sb[:, a0:a1], in_=wqT_t[:, a0:a1])

                # K for ALL blocks, chains interleaved at x-piece granularity
                # (8 PSUM banks) so the PE chases the x DMA with no idle
                with ExitStack() as s0ctx:
                    sp0 = s0ctx.enter_context(
                        tc.tile_pool(name=f"a_s0{rep}", bufs=1, space="PSUM"))
                    pk0 = [[sp0.tile([128, BLK], F32, tag=f"pk{b}{kvh}",
                                     name=f"pk0_{b}_{kvh}") for kvh in range(KV)]
                           for b in range(NBLK)]
                    for pc in range(len(pieces) - 1):
                        for a in range(pieces[pc], pieces[pc + 1]):
                            for b in range(NBLK):
                                for kvh in range(KV):
                                    nc.tensor.matmul(pk0[b][kvh][:],
                                                     wslice(wk_sb, a, kvh * HD, (kvh + 1) * HD),
                                                     xslice(a, b * BLK, (b + 1) * BLK),
                                                     start=(a == 0), stop=(a == NA - 1),
                                                     perf_mode=pm)
                    for b in range(NBLK):
                        for kvh in range(KV):
                            rope(kT_sb[:, kvh, b * BLK:(b + 1) * BLK],
                                 pk0[b][kvh][:], b * BLK, BLK)

                # V projections, then Q projections
                app = actx.enter_context(
                    tc.tile_pool(name=f"a_ps{rep}", bufs=3, space="PSUM"))
                for blk in range(NBLK):
                    t0 = blk * BLK
                    for tt in range(BLK // 128):
                        pv = app.tile([128, KV * HD], F32, tag="pv")
                        for a in range(NA):
                            nc.tensor.matmul(pv[:], xslice(a, t0 + tt * 128, t0 + (tt + 1) * 128),
                                             wslice(wv_sb, a, 0, KV * HD),
                                             start=(a == 0), stop=(a == NA - 1),
                                             perf_mode=pm)
                        nc.scalar.copy(v_sb[:, blk * (BLK // 128) + tt, :], pv[:])

                # ---- A2: Q projections ----
                for blk in range(NBLK):
                    t0 = blk * BLK
                    for h in range(HEADS):
                        pq = app.tile([128, BLK], F32, tag="pp")
                        for a in range(NA):
                            nc.tensor.matmul(pq[:], wslice(wq_sb, a, h * HD, (h + 1) * HD),
                                             xslice(a, t0, t0 + BLK),
                                             start=(a == 0), stop=(a == NA - 1),
                                             perf_mode=pm)
                        rope(qT_sb[:, h, t0:t0 + BLK], pq[:], t0, BLK)

            # ---------- phase BC: attention + o_proj, per q-chunk ----------
            with ExitStack() as bctx:
                bwo = bctx.enter_context(tc.tile_pool(name=f"c_w{rep}", bufs=1))
                wo_sb = bwo.tile([128, HEADS, HID], BF16)   # 25KB/part
                nc.sync.dma_start(out=wo_sb[:, 0:2, :], in_=woT_t[:, 0:2, :])
                nc.sync.dma_start(out=wo_sb[:, 2:HEADS, :], in_=woT_t[:, 2:HEADS, :])
                bsp = bctx.enter_context(tc.tile_pool(name=f"b_s{rep}", bufs=2, space="PSUM"))
                bap = bctx.enter_context(tc.tile_pool(name=f"b_at{rep}", bufs=2, space="PSUM"))
                rpp = bctx.enter_context(tc.tile_pool(name=f"b_rp{rep}", bufs=2, space="PSUM"))
                bep = bctx.enter_context(tc.tile_pool(name=f"b_e{rep}", bufs=3))
                esp = bctx.enter_context(tc.tile_pool(name=f"b_es{rep}", bufs=2))
                bwp = bctx.enter_context(tc.tile_pool(name=f"b_w{rep}", bufs=2))
                atp = bctx.enter_context(tc.tile_pool(name=f"b_atc{rep}", bufs=2))
                ycp = bctx.enter_context(tc.tile_pool(name=f"c_y{rep}", bufs=1))

                # o_proj for chunk cp, emitted as 5 slices of 4 (tt,n)-tiles
                # interleaved into the next chunk's head loop so the PE stays
                # fed while ACT works through that chunk's exps
                ystate = {}

                def oproj_slice(cp, at_prev, h):
                    if h == 0:
                        ystate["y"] = ycp.tile([128, 4, HID], BF16, tag="ych",
                                               name="y_ch")
                    y_ch = ystate["y"]
                    for k in range(4 * h, 4 * h + 4):
                        tt, n = divmod(k, NO)
                        py = rpp.tile([128, CH], F32, tag="rp", name="py")
                        for hh in range(HEADS):
                            nc.tensor.matmul(py[:], at_prev[:, hh, tt * 128:(tt + 1) * 128],
                                             wo_sb[:, hh, n * CH:(n + 1) * CH],
                                             start=(hh == 0), stop=(hh == HEADS - 1))
                        nc.vector.tensor_copy(y_ch[:, tt, n * CH:(n + 1) * CH], py[:])
                        if n == NO - 1:
                            nc.sync.dma_start(out=y_r[:, cp, tt, :], in_=y_ch[:, tt, :])

                at_prev = None
                for c in range(NCH):
                    q0 = c * CH
                    ki_max = 4 * c + 3
                    at_ch = atp.tile([128, HEADS, CH], BF16, tag="atc")  # 5KB/part
                    for h in range(HEADS):
                        kvh = KVIDX[h]
                        pat = bap.tile([128, CH], F32, tag="pat")
                        esum = esp.tile([128, CH], F16, tag="esum")
                        # off-diagonal k-tiles, exp'd in 1024-wide pairs
                        for kp in range(2 * c):
                            ps = bsp.tile([128, 2 * CH], F32, tag="ps")
                            for j in range(2):
                                ki = 2 * kp + j
                                nc.tensor.matmul(ps[:, j * CH:(j + 1) * CH],
                                                 kT_sb[:, kvh, ki * 128:(ki + 1) * 128],
                                                 qT_sb[:, h, q0:q0 + CH],
                                                 start=True, stop=True)
                            et = bep.tile([128, 2 * CH], BF16, tag="et")
                            nc.scalar.activation(out=et[:], in_=ps[:],
                                                 func=mybir.ActivationFunctionType.Exp,
                                                 scale=SCALE)
                            for j in range(2):
                                ki = 2 * kp + j
                                nc.tensor.matmul(pat[:], v_sb[:, ki, kvh * HD:(kvh + 1) * HD],
                                                 et[:, j * CH:(j + 1) * CH],
                                                 start=(ki == 0), stop=False)
                            # denominator: pair-add on DVE, accumulate fp16
                            # (sums < 5e3 << fp16 max; 2-byte dtype = fast DVE)
                            if kp == 0:
                                nc.vector.tensor_add(esum[:], et[:, 0:CH], et[:, CH:2 * CH])
                            else:
                                tmp = esp.tile([128, CH], F16, tag="tmp")
                                nc.vector.tensor_add(tmp[:], et[:, 0:CH], et[:, CH:2 * CH])
                                nc.vector.tensor_add(esum[:], esum[:], tmp[:])
                        # diagonal band k-tiles (4c .. 4c+3)
                        for ki in range(4 * c, ki_max + 1):
                            ps = bsp.tile([128, 2 * CH], F32, tag="ps")
                            off = ki * 128 - q0
                            nc.tensor.matmul(ps[:, off:CH],
                                             kT_sb[:, kvh, ki * 128:(ki + 1) * 128],
                                             qT_sb[:, h, q0 + off:q0 + CH],
                                             start=True, stop=True)
                            et = bep.tile([128, 2 * CH], BF16, tag="et")
                            nc.scalar.activation(out=et[:, off:CH], in_=ps[:, off:CH],
                                                 func=mybir.ActivationFunctionType.Exp,
                                                 scale=SCALE)
                            nc.vector.tensor_mul(et[:, off:off + 128],
                                                 et[:, off:off + 128], mask_sb[:])
                            nc.tensor.matmul(pat[:, off:CH],
                                             v_sb[:, ki, kvh * HD:(kvh + 1) * HD],
                                             et[:, off:CH], start=(ki == 0),
                                             stop=(ki == ki_max))
                            if c == 0 and ki == 0:
                                nc.vector.tensor_copy(esum[:], et[:, 0:CH])
                            else:
                                nc.vector.tensor_add(esum[:, off:CH], esum[:, off:CH],
                                                     et[:, off:CH])
                        # o_proj slice of the previous chunk first: its PE work
                        # hides the DVE esum tail this head just queued
                        if c > 0:
                            oproj_slice(c - 1, at_prev, h)
                        # partition-reduce the fp16 esum with one ones-matmul
                        pR = rpp.tile([128, CH], F32, tag="rp")
                        nc.tensor.matmul(pR[:], ones_sb[:], esum[:], start=True, stop=True)
                        rec = bwp.tile([128, CH], F32, tag="rec")
                        nc.vector.reciprocal(rec[:], pR[:])
                        nc.vector.tensor_mul(at_ch[:, h, :], pat[:], rec[:])
                    at_prev = at_ch
                for h in range(HEADS):
                    oproj_slice(NCH - 1, at_prev, h)

    _split_waits(nc)
    nc.finalize()
    return nc


def core_heads(g):
    """Query-head and kv-head global indices for core group g (= core % 4)."""
    qh = [4 * g, 4 * g + 1, 4 * g + 2, 4 * g + 3, 16 + g]
    kvh = [g, 4]
    return qh, kvh


def make_in_maps(hidden_states, position_ids, wq, wk, wv, wo, sub_w):
    hidden_states = np.asarray(hidden_states, dtype=np.float32)
    position_ids = np.asarray(position_ids)
    wq = np.asarray(wq, dtype=np.float32)
    wk = np.asarray(wk, dtype=np.float32)
    wv = np.asarray(wv, dtype=np.float32)
    wo = np.asarray(wo, dtype=np.float32)
    sub_w = np.asarray(sub_w, dtype=np.float32)

    wo_s = wo * sub_w[None, :]          # fold BitNetSubNorm gain into o_proj
    inv_freq = (1.0 / (THETA ** (np.arange(0, HD, 2, dtype=np.float32) / HD)))  # [64]
    mask01 = np.triu(np.ones((128, 128))).astype(ml_dtypes.bfloat16)

    bf = ml_dtypes.bfloat16
    if FP8_PROJ:
        f8 = mybir.dt.np(FP8)
        xsc, wsc, isc = X_SCALE, W_SCALE, 1.0 / (X_SCALE * W_SCALE)

        def cvt(m, s):
            return np.clip(m * s, -240.0, 240.0).astype(f8)
    else:
        xsc, wsc, isc = 1.0, 1.0, 1.0

        def cvt(m, s):
            return m.astype(bf)

    in_maps = []
    for c in range(NCORES):
        b, g = c // 4, c % 4
        qh, kvh = core_heads(g)
        qrows = np.concatenate([np.arange(h * HD, (h + 1) * HD) for h in qh])
        krows = np.concatenate([np.arange(k * HD, (k + 1) * HD) for k in kvh])

        pos = position_ids[b].astype(np.float32)                      # [S]
        ang = inv_freq[:, None] * pos[None, :]                        # [64, S]
        cosT = np.concatenate([np.cos(ang), np.cos(ang)], axis=0)     # [128, S]
        sinT = np.concatenate([-np.sin(ang), np.sin(ang)], axis=0)    # sign-folded

        in_maps.append({
            "xT": cvt(np.ascontiguousarray(hidden_states[b].T), xsc),  # [HID, S]
            "wqT": cvt(np.ascontiguousarray(wq[qrows].T), wsc),        # [HID, 640]
            "wkT": cvt(np.ascontiguousarray(wk[krows].T), wsc),        # [HID, 256]
            "wvT": cvt(np.ascontiguousarray(wv[krows].T), wsc),        # [HID, 256]
            "woT": np.ascontiguousarray(wo_s[:, qrows].T * isc).astype(bf),
            "cosT": np.ascontiguousarray(cosT * isc).astype(bf),
            "sinT": np.ascontiguousarray(sinT * isc).astype(bf),
            "mask": mask01,
            "ones": np.ones((128, 128), dtype=bf),
        })
    return in_maps


def kernel(hidden_states, position_ids, wq, wk, wv, wo, sub_w, _trace=False):
    if "nc" not in _CACHE:
        _CACHE["nc"] = build_nc()
    nc = _CACHE["nc"]
    in_maps = make_in_maps(hidden_states, position_ids, wq, wk, wv, wo, sub_w)
    res = run_bass_kernel_spmd(nc, in_maps, core_ids=list(range(NCORES)), trace=_trace)
    _CACHE["last_results"] = res
    out = np.zeros((B, S, HID), dtype=np.float32)
    for c in range(NCORES):
        out[c // 4] += res.results[c]["y"].astype(np.float32)
    return out


# revision 40
# speedup vs baseline: 1.6585x; 1.6585x over previous
"""BitNetAttention Trainium2 kernel (nn_BitNetAttention, B=2 S=2048 HID=2560).

Reference: q/k/v projections (x @ W^T), RoPE (rotate-half, theta=5e5), causal
GQA attention (20 q heads, 5 kv heads, head_dim 128), BitNetSubNorm per-channel
gain, o_proj.

Sharding across 8 NeuronCores: core c handles batch c//4 and 5 query heads:
with g = c%4, q heads [4g..4g+3, 16+g], kv heads [g, 4]. This grouping makes
the local head->kv map the constant [0,0,0,0,1] so one SPMD program serves all
cores. Each core computes its batch's partial o_proj output (sum over its 5
heads); the host sums 4 partials per batch. sub_w is folded into wo on host.

Design (all-bf16 data, fp32 PSUM accumulation; measured ~285us/rep on HW,
rel err 3.4e-3 vs the fp32 reference):
  - x, wq, wk, wv, wo, cos/sin staged bf16; x resident in SBUF for the whole
    A phase (loaded once). DMA is issued piece-by-piece (weights interleaved
    with x hid-slices) so the K chains chase the loads with no PE idle.
  - A1-K: the 8 K accumulation chains (4 t-blocks x 2 kv heads) interleave at
    x-piece granularity across all 8 PSUM banks. RoPE: ACT drains PSUM via
    straight + rotate-half bf16 copies (sign folded into sinT), then DVE
    multiplies with the bf16 cos/sin tables and adds (all 2-byte SBUF ops).
  - A1-V/A2-Q: straight 20-link chains; V copies to [t,k-tile,d] bf16 on ACT.
  - BC: per (512-wide q-chunk, head): QK^T in sT layout [k,q] (bf16) with
    double-buffered score PSUM, exp on ACT (scale=1/sqrt(128), no max
    subtraction -- scores are O(1) gaussians; 1024-wide paired exp off the
    diagonal), causal triangular 0/1 mask on diagonal tiles, PV accumulating
    in PSUM over k-tiles. Softmax denominators: DVE pair-adds of exp tiles
    into an fp16 esum (fast 2-byte DVE mode; sums << fp16 max), one
    ones-matmul partition-reduce per (head, chunk), reciprocal+mul.
  - o_proj of chunk c-1 is emitted in 4-tile slices INSIDE chunk c's head
    loop: its PE work fills the PE while ACT works through that chunk's exps
    (the attention sections alone are exp-bound). y written bf16, one DMA
    per 128-row group; host sums the 4 partials per batch in fp32.
Tried and rejected on HW measurement: fp8 E4M3 DoubleRow projections (slower
than bf16 -- 329us vs 281us -- and rel err 3.2e-2 > the 2e-2 gate), QK
moving-side head pairing (walrus codegen rejects 3D strided moving APs),
deeper tile pools (app 4 / bep 4 / esp 3 / ycp 2 regressed to 372us).
"""

import numpy as np
import ml_dtypes
from contextlib import ExitStack

import concourse.bass as bass
import concourse.mybir as mybir
import concourse.tile as tile
from concourse.bass_utils import run_bass_kernel_spmd

F32 = mybir.dt.float32
F16 = mybir.dt.float16
BF16 = mybir.dt.bfloat16
FP8 = mybir.dt.float8e4
DR = mybir.MatmulPerfMode.DoubleRow

# fp8 (E4M3) DoubleRow projections. Measured on HW: SLOWER than bf16 (329us
# vs 281us) and rel err 3.2e-2 > 2e-2 gate. Keep off; code path retained.
FP8_PROJ = False
X_SCALE = 8.0
W_SCALE = 64.0

B, S, HID = 2, 2048, 2560
NH, NKV, HD = 20, 5, 128
G = NH // NKV
THETA = 500000.0
NCORES = 8
HEADS = 5          # query heads per core
KV = 2             # kv heads per core
KVIDX = [0, 0, 0, 0, 1]   # local head -> local kv head
HT = HID // 128    # 20 hidden k-tiles
BLK = 512          # t block width for projections
NBLK = S // BLK    # 4
CH = 512           # attention q-chunk width
NCH = S // CH      # 4
KT = S // 128      # 16 k-tiles
NO = HID // CH     # 5 o_proj hid chunks
SCALE = HD ** -0.5

_CACHE = {}


def _split_waits(nc):
    """Walrus ISA structs carry a single sync-wait slot. Move surplus waits
    onto EventSemaphore sequencer instructions inserted just before (same
    engine; engines are in-order so hoisting waits earlier is safe)."""
    import concourse.mybir as mb
    n_ev = 0
    for f in nc.m.functions:
        for bb in f.blocks:
            out = []
            changed = False
            for inst in bb.instructions:
                si = getattr(inst, "sync_info", None)
                if (type(inst).__name__ != "InstEventSemaphore" and si is not None
                        and len(si.on_wait) > 1):
                    waits = list(si.on_wait)
                    for w in waits[:-1]:
                        ev = mb.InstEventSemaphore(name=f"I-evw-{n_ev}", ins=[], outs=[])
                        n_ev += 1
                        ev.engine = inst.engine
                        ev.sync_info = mb.SyncInfo(on_wait=[w], on_update=[])
                        nc.register_instruction(ev)
                        out.append(ev)
                    inst.sync_info = mb.SyncInfo(on_wait=waits[-1:],
                                                 on_update=list(si.on_update))
                    changed = True
                out.append(inst)
            if changed:
                bb.instructions = out
    return n_ev


def build_nc(reps=1, a_only=False):
    # a_only=True: projections + RoPE only (timing ablation; y stays zero)
    nc = bass.Bass()
    xdt = FP8 if FP8_PROJ else BF16
    xT = nc.declare_dram_parameter("xT", [HID, S], xdt, isOutput=False)
    wqT = nc.declare_dram_parameter("wqT", [HID, HEADS * HD], xdt, isOutput=False)
    wkT = nc.declare_dram_parameter("wkT", [HID, KV * HD], xdt, isOutput=False)
    wvT = nc.declare_dram_parameter("wvT", [HID, KV * HD], xdt, isOutput=False)
    woT = nc.declare_dram_parameter("woT", [HEADS * HD, HID], BF16, isOutput=False)
    cosT = nc.declare_dram_parameter("cosT", [HD, S], BF16, isOutput=False)
    sinT = nc.declare_dram_parameter("sinT", [HD, S], BF16, isOutput=False)  # sign-folded
    mask = nc.declare_dram_parameter("mask", [128, 128], BF16, isOutput=False)  # triu 0/1
    ones = nc.declare_dram_parameter("ones", [128, 128], BF16, isOutput=False)
    y = nc.declare_dram_parameter("y", [S, HID], BF16, isOutput=True)

    if FP8_PROJ:
        # hid = 256*a + 128*j + p: 3D [p, 2, free] APs for DoubleRow matmuls
        xT_t = xT.rearrange("(a j p) t -> p a j t", p=128, j=2)    # [128, 10, 2, 2048]
        wqT_t = wqT.rearrange("(a j p) d -> p a j d", p=128, j=2)  # [128, 10, 2, 640]
        wkT_t = wkT.rearrange("(a j p) d -> p a j d", p=128, j=2)
        wvT_t = wvT.rearrange("(a j p) d -> p a j d", p=128, j=2)
    else:
        xT_t = xT.rearrange("(a p) t -> p a t", p=128)      # [128, 20, 2048]
        wqT_t = wqT.rearrange("(a p) d -> p a d", p=128)    # [128, 20, 640]
        wkT_t = wkT.rearrange("(a p) d -> p a d", p=128)
        wvT_t = wvT.rearrange("(a p) d -> p a d", p=128)
    woT_t = woT.rearrange("(h p) n -> p h n", p=128)    # [128, 5, 2560]
    y_r = y.rearrange("(c tt p) n -> p c tt n", p=128, tt=4)  # [128, 4, 4, 2560]

    with tile.TileContext(nc) as tc:
      for rep in range(reps):
        with ExitStack() as ctx:
            # ---------- persistent tiles (live through BC) ----------
            per = ctx.enter_context(tc.tile_pool(name=f"persist{rep}", bufs=1))
            kT_sb = per.tile([128, KV, S], BF16)        # 8KB/part
            v_sb = per.tile([128, KT, KV * HD], BF16)   # 8KB/part [t%128, ktile, kv*128+d]
            qT_sb = per.tile([128, HEADS, S], BF16)     # 20KB/part
            mask_sb = per.tile([128, 128], BF16)
            ones_sb = per.tile([128, 128], BF16)
            nc.sync.dma_start(out=mask_sb, in_=mask[:])
            nc.sync.dma_start(out=ones_sb, in_=ones[:])

            # ---------- phase A: projections (x resident) ----------
            with ExitStack() as actx:
                axp = actx.enter_context(tc.tile_pool(name=f"a_x{rep}", bufs=1))
                if FP8_PROJ:
                    x_sb = axp.tile([128, HT // 2, 2, S], FP8)  # 40KB/part
                else:
                    x_sb = axp.tile([128, HT, S], BF16)  # 80KB/part
                cos_sb = axp.tile([HD, S], BF16)         # 4KB
                sin_sb = axp.tile([HD, S], BF16)         # 4KB
                ropep = actx.enter_context(tc.tile_pool(name=f"a_rope{rep}", bufs=2))

                def rope(dst, psrc, t0, w):
                    # ACT drains the PSUM bank fast (straight + rotate-half
                    # copies; partition-offset reads legal from PSUM); DVE
                    # then works all-bf16 SBUF with aligned base partitions
                    pb = ropep.tile([128, BLK], BF16, tag="pb")
                    pbr = ropep.tile([128, BLK], BF16, tag="pbr")
                    nc.scalar.copy(pb[:, 0:w], psrc)
                    nc.scalar.copy(pbr[0:64, 0:w], psrc[64:128, :])
                    nc.scalar.copy(pbr[64:128, 0:w], psrc[0:64, :])
                    t1 = ropep.tile([128, BLK], BF16, tag="t1")
                    t2 = ropep.tile([128, BLK], BF16, tag="t2")
                    nc.vector.tensor_mul(t1[:, 0:w], pb[:, 0:w], cos_sb[:, t0:t0 + w])
                    nc.vector.tensor_mul(t2[:, 0:w], pbr[:, 0:w], sin_sb[:, t0:t0 + w])
                    nc.vector.tensor_add(dst, t1[:, 0:w], t2[:, 0:w])

                # ---- A1: K/V projections ----
                NA = HT // 2 if FP8_PROJ else HT   # contraction tiles
                pm = DR if FP8_PROJ else None
                awp = actx.enter_context(tc.tile_pool(name=f"a_w{rep}", bufs=1))
                awq = actx.enter_context(tc.tile_pool(name=f"q_w{rep}", bufs=1))
                if FP8_PROJ:
                    wk_sb = awp.tile([128, NA, 2, KV * HD], FP8)     # 5KB/part
                    wv_sb = awp.tile([128, NA, 2, KV * HD], FP8)     # 5KB/part
                    wq_sb = awq.tile([128, NA, 2, HEADS * HD], FP8)  # 12.5KB/part
                    pieces = [0, 1, 2, 4, 6, 8, NA]
                else:
                    wk_sb = awp.tile([128, NA, KV * HD], BF16)   # 10KB/part
                    wv_sb = awp.tile([128, NA, KV * HD], BF16)   # 10KB/part
                    wq_sb = awq.tile([128, NA, HEADS * HD], BF16)  # 25KB/part
                    pieces = [0, 2, 4, 8, 12, 16, NA]

                def wslice(w_sb, a, d0, d1):
                    return w_sb[:, a, :, d0:d1] if FP8_PROJ else w_sb[:, a, d0:d1]

                def xslice(a, t0, t1):
                    return x_sb[:, a, :, t0:t1] if FP8_PROJ else x_sb[:, a, t0:t1]

                # interleave weight and x loads piece-by-piece: the shared DMA
                # path delivers exactly what the K chains need next
                for pc in range(len(pieces) - 1):
                    a0, a1 = pieces[pc], pieces[pc + 1]
                    nc.sync.dma_start(out=wk_sb[:, a0:a1], in_=wkT_t[:, a0:a1])
                    nc.sync.dma_start(out=x_sb[:, a0:a1], in_=xT_t[:, a0:a1])
                    if pc == 1:
                        nc.sync.dma_start(out=cos_sb, in_=cosT[:])
                        nc.sync.dma_start(out=sin_sb, in_=sinT[:])
                for pc in range(len(pieces) - 1):
                    a0, a1 = pieces[pc], pieces[pc + 1]
                    nc.sync.dma_start(out=wv_sb[:, a0:a1], in_=wvT_t[:, a0:a1])
                for pc in range(len(pieces) - 1):
                    a0, a1 = pieces[pc], pieces[pc + 1]
                    nc.sync.dma_start(out=wq_sb[:, a0:a1], in_=wqT_t[:, a0:a1])

                # K for ALL blocks, chains interleaved at x-piece granularity
                # (8 PSUM banks) so the PE chases the x DMA with no idle
                with ExitStack() as s0ctx:
                    sp0 = s0ctx.enter_context(
                        tc.tile_pool(name=f"a_s0{rep}", bufs=1, space="PSUM"))
                    pk0 = [[sp0.tile([128, BLK], F32, tag=f"pk{b}{kvh}",
                                     name=f"pk0_{b}_{kvh}") for kvh in range(KV)]
                           for b in range(NBLK)]
                    for pc in range(len(pieces) - 1):
                        for a in range(pieces[pc], pieces[pc + 1]):
                            for b in range(NBLK):
                                for kvh in range(KV):
                                    nc.tensor.matmul(pk0[b][kvh][:],
                                                     wslice(wk_sb, a, kvh * HD, (kvh + 1) * HD),
                                                     xslice(a, b * BLK, (b + 1) * BLK),
                                                     start=(a == 0), stop=(a == NA - 1),
                                                     perf_mode=pm)
                    for b in range(NBLK):
                        for kvh in range(KV):
                            rope(kT_sb[:, kvh, b * BLK:(b + 1) * BLK],
                                 pk0[b][kvh][:], b * BLK, BLK)

                # V projections, then Q projections
                app = actx.enter_context(
                    tc.tile_pool(name=f"a_ps{rep}", bufs=3, space="PSUM"))
                for blk in range(NBLK):
                    t0 = blk * BLK
                    for tt in range(BLK // 128):
                        pv = app.tile([128, KV * HD], F32, tag="pv")
                        for a in range(NA):
                            nc.tensor.matmul(pv[:], xslice(a, t0 + tt * 128, t0 + (tt + 1) * 128),
                                             wslice(wv_sb, a, 0, KV * HD),
                                             start=(a == 0), stop=(a == NA - 1),
                                             perf_mode=pm)
                        nc.scalar.copy(v_sb[:, blk * (BLK // 128) + tt, :], pv[:])

                # ---- A2: Q projections ----
                for blk in range(NBLK):
                    t0 = blk * BLK
                    for h in range(HEADS):
                        pq = app.tile([128, BLK], F32, tag="pp")
                        for a in range(NA):
                            nc.tensor.matmul(pq[:], wslice(wq_sb, a, h * HD, (h + 1) * HD),
                                             xslice(a, t0, t0 + BLK),
                                             start=(a == 0), stop=(a == NA - 1),
                                             perf_mode=pm)
                        rope(qT_sb[:, h, t0:t0 + BLK], pq[:], t0, BLK)

            # ---------- phase BC: attention + o_proj, per q-chunk ----------
            if a_only:
                # keep the projection results live (defeat dead-code elim):
                # dump kT/qT/v into disjoint y rows
                for h in range(HEADS):
                    nc.sync.dma_start(out=y[h * 128:(h + 1) * 128, 0:S],
                                      in_=qT_sb[:, h, :])
                for kvh in range(KV):
                    nc.sync.dma_start(out=y[640 + kvh * 128:640 + (kvh + 1) * 128, 0:S],
                                      in_=kT_sb[:, kvh, :])
                nc.sync.dma_start(out=y[896:1024, 0:S], in_=v_sb[:, 0:8, :])
                nc.sync.dma_start(out=y[1024:1152, 0:S], in_=v_sb[:, 8:16, :])
                continue
            with ExitStack() as bctx:
                bwo = bctx.enter_context(tc.tile_pool(name=f"c_w{rep}", bufs=1))
                wo_sb = bwo.tile([128, HEADS, HID], BF16)   # 25KB/part
                nc.sync.dma_start(out=wo_sb[:, 0:2, :], in_=woT_t[:, 0:2, :])
                nc.sync.dma_start(out=wo_sb[:, 2:HEADS, :], in_=woT_t[:, 2:HEADS, :])
                bsp = bctx.enter_context(tc.tile_pool(name=f"b_s{rep}", bufs=2, space="PSUM"))
                bap = bctx.enter_context(tc.tile_pool(name=f"b_at{rep}", bufs=2, space="PSUM"))
                rpp = bctx.enter_context(tc.tile_pool(name=f"b_rp{rep}", bufs=2, space="PSUM"))
                bep = bctx.enter_context(tc.tile_pool(name=f"b_e{rep}", bufs=3))
                esp = bctx.enter_context(tc.tile_pool(name=f"b_es{rep}", bufs=2))
                bwp = bctx.enter_context(tc.tile_pool(name=f"b_w{rep}", bufs=2))
                atp = bctx.enter_context(tc.tile_pool(name=f"b_atc{rep}", bufs=2))
                ycp = bctx.enter_context(tc.tile_pool(name=f"c_y{rep}", bufs=1))

                # o_proj for chunk cp, emitted as 5 slices of 4 (tt,n)-tiles
                # interleaved into the next chunk's head loop so the PE stays
                # fed while ACT works through that chunk's exps
                ystate = {}

                def oproj_slice(cp, at_prev, h):
                    if h == 0:
                        ystate["y"] = ycp.tile([128, 4, HID], BF16, tag="ych",
                                               name="y_ch")
                    y_ch = ystate["y"]
                    for k in range(4 * h, 4 * h + 4):
                        tt, n = divmod(k, NO)
                        py = rpp.tile([128, CH], F32, tag="rp", name="py")
                        for hh in range(HEADS):
                            nc.tensor.matmul(py[:], at_prev[:, hh, tt * 128:(tt + 1) * 128],
                                             wo_sb[:, hh, n * CH:(n + 1) * CH],
                                             start=(hh == 0), stop=(hh == HEADS - 1))
                        # PSUM->SBUF copies split across ACT and DVE: both are
                        # loaded in BC (exp vs esum/normalize), share the cost
                        if k % 2 == 0:
                            nc.scalar.copy(y_ch[:, tt, n * CH:(n + 1) * CH], py[:])
                        else:
                            nc.vector.tensor_copy(y_ch[:, tt, n * CH:(n + 1) * CH], py[:])
                        if n == NO - 1:
                            nc.sync.dma_start(out=y_r[:, cp, tt, :], in_=y_ch[:, tt, :])

                at_prev = None
                for c in range(NCH):
                    q0 = c * CH
                    ki_max = 4 * c + 3
                    at_ch = atp.tile([128, HEADS, CH], BF16, tag="atc")  # 5KB/part
                    for h in range(HEADS):
                        kvh = KVIDX[h]
                        pat = bap.tile([128, CH], F32, tag="pat")
                        esum = esp.tile([128, CH], F16, tag="esum")
                        # off-diagonal k-tiles, exp'd in 1024-wide pairs
                        for kp in range(2 * c):
                            ps = bsp.tile([128, 2 * CH], F32, tag="ps")
                            for j in range(2):
                                ki = 2 * kp + j
                                nc.tensor.matmul(ps[:, j * CH:(j + 1) * CH],
                                                 kT_sb[:, kvh, ki * 128:(ki + 1) * 128],
                                                 qT_sb[:, h, q0:q0 + CH],
                                                 start=True, stop=True)
                            et = bep.tile([128, 2 * CH], BF16, tag="et")
                            nc.scalar.activation(out=et[:], in_=ps[:],
                                                 func=mybir.ActivationFunctionType.Exp,
                                                 scale=SCALE)
                            for j in range(2):
                                ki = 2 * kp + j
                                nc.tensor.matmul(pat[:], v_sb[:, ki, kvh * HD:(kvh + 1) * HD],
                                                 et[:, j * CH:(j + 1) * CH],
                                                 start=(ki == 0), stop=False)
                            # denominator: pair-add on DVE, accumulate fp16
                            # (sums < 5e3 << fp16 max; 2-byte dtype = fast DVE)
                            if kp == 0:
                                nc.vector.tensor_add(esum[:], et[:, 0:CH], et[:, CH:2 * CH])
                            else:
                                tmp = esp.tile([128, CH], F16, tag="tmp")
                                nc.vector.tensor_add(tmp[:], et[:, 0:CH], et[:, CH:2 * CH])
                                nc.vector.tensor_add(esum[:], esum[:], tmp[:])
                        # diagonal band k-tiles (4c .. 4c+3)
                        for ki in range(4 * c, ki_max + 1):
                            ps = bsp.tile([128, 2 * CH], F32, tag="ps")
                            off = ki * 128 - q0
                            nc.tensor.matmul(ps[:, off:CH],
                                             kT_sb[:, kvh, ki * 128:(ki + 1) * 128],
                                             qT_sb[:, h, q0 + off:q0 + CH],
                                             start=True, stop=True)
                            et = bep.tile([128, 2 * CH], BF16, tag="et")
                            nc.scalar.activation(out=et[:, off:CH], in_=ps[:, off:CH],
                                                 func=mybir.ActivationFunctionType.Exp,
                                                 scale=SCALE)
                            nc.vector.tensor_mul(et[:, off:off + 128],
                                                 et[:, off:off + 128], mask_sb[:])
                            nc.tensor.matmul(pat[:, off:CH],
                                             v_sb[:, ki, kvh * HD:(kvh + 1) * HD],
                                             et[:, off:CH], start=(ki == 0),
                                             stop=(ki == ki_max))
                            if c == 0 and ki == 0:
                                nc.vector.tensor_copy(esum[:], et[:, 0:CH])
                            else:
                                nc.vector.tensor_add(esum[:, off:CH], esum[:, off:CH],
                                                     et[:, off:CH])
                        # o_proj slice of the previous chunk first: its PE work
                        # hides the DVE esum tail this head just queued
                        if c > 0:
                            oproj_slice(c - 1, at_prev, h)
                        # partition-reduce the fp16 esum with one ones-matmul
                        pR = rpp.tile([128, CH], F32, tag="rp")
                        nc.tensor.matmul(pR[:], ones_sb[:], esum[:], start=True, stop=True)
                        rec = bwp.tile([128, CH], F32, tag="rec")
                        nc.vector.reciprocal(rec[:], pR[:])
                        nc.vector.tensor_mul(at_ch[:, h, :], pat[:], rec[:])
                    at_prev = at_ch
                for h in range(HEADS):
                    oproj_slice(NCH - 1, at_prev, h)

    _split_waits(nc)
    nc.finalize()
    return nc


def core_heads(g):
    """Query-head and kv-head global indices for core group g (= core % 4)."""
    qh = [4 * g, 4 * g + 1, 4 * g + 2, 4 * g + 3, 16 + g]
    kvh = [g, 4]
    return qh, kvh


def make_in_maps(hidden_states, position_ids, wq, wk, wv, wo, sub_w):
    hidden_states = np.asarray(hidden_states, dtype=np.float32)
    position_ids = np.asarray(position_ids)
    wq = np.asarray(wq, dtype=np.float32)
    wk = np.asarray(wk, dtype=np.float32)
    wv = np.asarray(wv, dtype=np.float32)
    wo = np.asarray(wo, dtype=np.float32)
    sub_w = np.asarray(sub_w, dtype=np.float32)

    wo_s = wo * sub_w[None, :]          # fold BitNetSubNorm gain into o_proj
    inv_freq = (1.0 / (THETA ** (np.arange(0, HD, 2, dtype=np.float32) / HD)))  # [64]
    mask01 = np.triu(np.ones((128, 128))).astype(ml_dtypes.bfloat16)

    bf = ml_dtypes.bfloat16
    if FP8_PROJ:
        f8 = mybir.dt.np(FP8)
        xsc, wsc, isc = X_SCALE, W_SCALE, 1.0 / (X_SCALE * W_SCALE)

        def cvt(m, s):
            return np.clip(m * s, -240.0, 240.0).astype(f8)
    else:
        xsc, wsc, isc = 1.0, 1.0, 1.0

        def cvt(m, s):
            return m.astype(bf)

    in_maps = []
    for c in range(NCORES):
        b, g = c // 4, c % 4
        qh, kvh = core_heads(g)
        qrows = np.concatenate([np.arange(h * HD, (h + 1) * HD) for h in qh])
        krows = np.concatenate([np.arange(k * HD, (k + 1) * HD) for k in kvh])

        pos = position_ids[b].astype(np.float32)                      # [S]
        ang = inv_freq[:, None] * pos[None, :]                        # [64, S]
        cosT = np.concatenate([np.cos(ang), np.cos(ang)], axis=0)     # [128, S]
        sinT = np.concatenate([-np.sin(ang), np.sin(ang)], axis=0)    # sign-folded

        in_maps.append({
            "xT": cvt(np.ascontiguousarray(hidden_states[b].T), xsc),  # [HID, S]
            "wqT": cvt(np.ascontiguousarray(wq[qrows].T), wsc),        # [HID, 640]
            "wkT": cvt(np.ascontiguousarray(wk[krows].T), wsc),        # [HID, 256]
            "wvT": cvt(np.ascontiguousarray(wv[krows].T), wsc),        # [HID, 256]
            "woT": np.ascontiguousarray(wo_s[:, qrows].T * isc).astype(bf),
            "cosT": np.ascontiguousarray(cosT * isc).astype(bf),
            "sinT": np.ascontiguousarray(sinT * isc).astype(bf),
            "mask": mask01,
            "ones": np.ones((128, 128), dtype=bf),
        })
    return in_maps


def kernel(hidden_states, position_ids, wq, wk, wv, wo, sub_w, _trace=False):
    if "nc" not in _CACHE:
        _CACHE["nc"] = build_nc()
    nc = _CACHE["nc"]
    in_maps = make_in_maps(hidden_states, position_ids, wq, wk, wv, wo, sub_w)
    res = run_bass_kernel_spmd(nc, in_maps, core_ids=list(range(NCORES)), trace=_trace)
    _CACHE["last_results"] = res
    out = np.zeros((B, S, HID), dtype=np.float32)
    for c in range(NCORES):
        out[c // 4] += res.results[c]["y"].astype(np.float32)
    return out


# revision 48
# speedup vs baseline: 2.1443x; 1.2929x over previous
"""BitNetAttention Trainium2 kernel (nn_BitNetAttention, B=2 S=2048 HID=2560).

Reference: q/k/v projections (x @ W^T), RoPE (rotate-half, theta=5e5), causal
GQA attention (20 q heads, 5 kv heads, head_dim 128), BitNetSubNorm per-channel
gain, o_proj.

Sharding across 8 NeuronCores: core c handles batch c//4 and 5 query heads:
with g = c%4, q heads [4g..4g+3, 16+g], kv heads [g, 4]. This grouping makes
the local head->kv map the constant [0,0,0,0,1] so one SPMD program serves all
cores. Each core computes its batch's partial o_proj output (sum over its 5
heads); the host sums 4 partials per batch. sub_w is folded into wo on host.

Design (all-bf16 data, fp32 PSUM accumulation; measured ~285us/rep on HW,
rel err 3.4e-3 vs the fp32 reference):
  - x, wq, wk, wv, wo, cos/sin staged bf16; x resident in SBUF for the whole
    A phase (loaded once). DMA is issued piece-by-piece (weights interleaved
    with x hid-slices) so the K chains chase the loads with no PE idle.
  - A1-K: the 8 K accumulation chains (4 t-blocks x 2 kv heads) interleave at
    x-piece granularity across all 8 PSUM banks. RoPE: ACT drains PSUM via
    straight + rotate-half bf16 copies (sign folded into sinT), then DVE
    multiplies with the bf16 cos/sin tables and adds (all 2-byte SBUF ops).
  - A1-V/A2-Q: straight 20-link chains; V copies to [t,k-tile,d] bf16 on ACT.
  - BC: per (512-wide q-chunk, head): QK^T in sT layout [k,q] (bf16) with
    double-buffered score PSUM, exp on ACT (scale=1/sqrt(128), no max
    subtraction -- scores are O(1) gaussians; 1024-wide paired exp off the
    diagonal), causal triangular 0/1 mask on diagonal tiles, PV accumulating
    in PSUM over k-tiles. Softmax denominators: DVE pair-adds of exp tiles
    into an fp16 esum (fast 2-byte DVE mode; sums << fp16 max), one
    ones-matmul partition-reduce per (head, chunk), reciprocal+mul.
  - o_proj of chunk c-1 is emitted in 4-tile slices INSIDE chunk c's head
    loop: its PE work fills the PE while ACT works through that chunk's exps
    (the attention sections alone are exp-bound). o_proj PSUM->SBUF copies
    alternate between ACT and DVE; y written bf16, one DMA per 128-row
    group; host sums the 4 partials per batch in fp32.
HW phase ablation (delta-timed, outputs kept live to defeat DCE): A phase
(projections+RoPE) ~81us/rep, BC (attention+o_proj) ~233us/rep -- which
implies the PE runs ~2 bf16 MACs/PE/cycle, so BC is bound by exp on ACT
(~100us) plus dependency-chain latency, not by PE.
Tried and rejected on HW measurement: fp8 E4M3 DoubleRow projections (slower
than bf16 -- 329us vs 281us -- and rel err 3.2e-2 > the 2e-2 gate), QK
moving-side head pairing (walrus codegen rejects 3D strided moving APs),
deeper tile pools (app 4 / bep 4 / esp 3 / ycp 2 regressed to 372us),
o_proj DMA directly from PSUM (dma_start cannot read PSUM).
"""

import numpy as np
import ml_dtypes
from contextlib import ExitStack

import concourse.bass as bass
import concourse.mybir as mybir
import concourse.tile as tile
from concourse.bass_utils import run_bass_kernel_spmd

F32 = mybir.dt.float32
F16 = mybir.dt.float16
BF16 = mybir.dt.bfloat16
FP8 = mybir.dt.float8e4
DR = mybir.MatmulPerfMode.DoubleRow

# fp8 (E4M3) DoubleRow projections. Measured on HW: SLOWER than bf16 (329us
# vs 281us) and rel err 3.2e-2 > 2e-2 gate. Keep off; code path retained.
FP8_PROJ = False
X_SCALE = 8.0
W_SCALE = 64.0

B, S, HID = 2, 2048, 2560
NH, NKV, HD = 20, 5, 128
G = NH // NKV
THETA = 500000.0
NCORES = 8
HEADS = 5          # query heads per core
KV = 2             # kv heads per core
KVIDX = [0, 0, 0, 0, 1]   # local head -> local kv head
HT = HID // 128    # 20 hidden k-tiles
BLK = 512          # t block width for projections
NBLK = S // BLK    # 4
CH = 512           # attention q-chunk width
NCH = S // CH      # 4
KT = S // 128      # 16 k-tiles
NO = HID // CH     # 5 o_proj hid chunks
SCALE = HD ** -0.5

_CACHE = {}


def _split_waits(nc):
    """Walrus ISA structs carry a single sync-wait slot. Move surplus waits
    onto EventSemaphore sequencer instructions inserted just before (same
    engine; engines are in-order so hoisting waits earlier is safe)."""
    import concourse.mybir as mb
    n_ev = 0
    for f in nc.m.functions:
        for bb in f.blocks:
            out = []
            changed = False
            for inst in bb.instructions:
                si = getattr(inst, "sync_info", None)
                if (type(inst).__name__ != "InstEventSemaphore" and si is not None
                        and len(si.on_wait) > 1):
                    waits = list(si.on_wait)
                    for w in waits[:-1]:
                        ev = mb.InstEventSemaphore(name=f"I-evw-{n_ev}", ins=[], outs=[])
                        n_ev += 1
                        ev.engine = inst.engine
                        ev.sync_info = mb.SyncInfo(on_wait=[w], on_update=[])
                        nc.register_instruction(ev)
                        out.append(ev)
                    inst.sync_info = mb.SyncInfo(on_wait=waits[-1:],
                                                 on_update=list(si.on_update))
                    changed = True
                out.append(inst)
            if changed:
                bb.instructions = out
    return n_ev


def build_nc(reps=1, a_only=False):
    # a_only=True: projections + RoPE only (timing ablation; y stays zero)
    nc = bass.Bass()
    xdt = FP8 if FP8_PROJ else BF16
    xT = nc.declare_dram_parameter("xT", [HID, S], xdt, isOutput=False)
    wqT = nc.declare_dram_parameter("wqT", [HID, HEADS * HD], xdt, isOutput=False)
    wkT = nc.declare_dram_parameter("wkT", [HID, KV * HD], xdt, isOutput=False)
    wvT = nc.declare_dram_parameter("wvT", [HID, KV * HD], xdt, isOutput=False)
    woT = nc.declare_dram_parameter("woT", [HEADS * HD, HID], BF16, isOutput=False)
    cosT = nc.declare_dram_parameter("cosT", [HD, S], BF16, isOutput=False)
    sinT = nc.declare_dram_parameter("sinT", [HD, S], BF16, isOutput=False)  # sign-folded
    mask = nc.declare_dram_parameter("mask", [128, 128], BF16, isOutput=False)  # triu 0/1
    ones = nc.declare_dram_parameter("ones", [128, 128], BF16, isOutput=False)
    y = nc.declare_dram_parameter("y", [S, HID], BF16, isOutput=True)

    if FP8_PROJ:
        # hid = 256*a + 128*j + p: 3D [p, 2, free] APs for DoubleRow matmuls
        xT_t = xT.rearrange("(a j p) t -> p a j t", p=128, j=2)    # [128, 10, 2, 2048]
        wqT_t = wqT.rearrange("(a j p) d -> p a j d", p=128, j=2)  # [128, 10, 2, 640]
        wkT_t = wkT.rearrange("(a j p) d -> p a j d", p=128, j=2)
        wvT_t = wvT.rearrange("(a j p) d -> p a j d", p=128, j=2)
    else:
        xT_t = xT.rearrange("(a p) t -> p a t", p=128)      # [128, 20, 2048]
        wqT_t = wqT.rearrange("(a p) d -> p a d", p=128)    # [128, 20, 640]
        wkT_t = wkT.rearrange("(a p) d -> p a d", p=128)
        wvT_t = wvT.rearrange("(a p) d -> p a d", p=128)
    woT_t = woT.rearrange("(h p) n -> p h n", p=128)    # [128, 5, 2560]
    y_r = y.rearrange("(c tt p) n -> p c tt n", p=128, tt=4)  # [128, 4, 4, 2560]

    with tile.TileContext(nc) as tc:
      for rep in range(reps):
        with ExitStack() as ctx:
            # ---------- persistent tiles (live through BC) ----------
            per = ctx.enter_context(tc.tile_pool(name=f"persist{rep}", bufs=1))
            kT_sb = per.tile([128, KV, S], BF16)        # 8KB/part
            v_sb = per.tile([128, KT, KV * HD], BF16)   # 8KB/part [t%128, ktile, kv*128+d]
            qT_sb = per.tile([128, HEADS, S], BF16)     # 20KB/part
            cos_sb = per.tile([HD, S], BF16)            # 4KB
            sin_sb = per.tile([HD, S], BF16)            # 4KB
            mask_sb = per.tile([128, 128], BF16)
            ones_sb = per.tile([128, 128], BF16)
            nc.sync.dma_start(out=mask_sb, in_=mask[:])
            nc.sync.dma_start(out=ones_sb, in_=ones[:])
            # rope tools + wq live through BC: Q-projection chains for block
            # c+1 are interleaved into BC chunk c (BC is exp-bound, not
            # PE-bound, so they ride in the PE's stall gaps)
            ropep = ctx.enter_context(tc.tile_pool(name=f"a_rope{rep}", bufs=2))
            awq = ctx.enter_context(tc.tile_pool(name=f"q_w{rep}", bufs=1))

            def rope(dst, psrc, t0, w):
                # ACT drains the PSUM bank fast (straight + rotate-half
                # copies; partition-offset reads legal from PSUM); DVE
                # then works all-bf16 SBUF with aligned base partitions
                pb = ropep.tile([128, BLK], BF16, tag="pb")
                pbr = ropep.tile([128, BLK], BF16, tag="pbr")
                nc.scalar.copy(pb[:, 0:w], psrc)
                nc.scalar.copy(pbr[0:64, 0:w], psrc[64:128, :])
                nc.scalar.copy(pbr[64:128, 0:w], psrc[0:64, :])
                t1 = ropep.tile([128, BLK], BF16, tag="t1")
                t2 = ropep.tile([128, BLK], BF16, tag="t2")
                nc.vector.tensor_mul(t1[:, 0:w], pb[:, 0:w], cos_sb[:, t0:t0 + w])
                nc.vector.tensor_mul(t2[:, 0:w], pbr[:, 0:w], sin_sb[:, t0:t0 + w])
                nc.vector.tensor_add(dst, t1[:, 0:w], t2[:, 0:w])

            # ---------- phase A: projections (x resident) ----------
            with ExitStack() as actx:
                axp = actx.enter_context(tc.tile_pool(name=f"a_x{rep}", bufs=1))
                if FP8_PROJ:
                    x_sb = axp.tile([128, HT // 2, 2, S], FP8)  # 40KB/part
                else:
                    x_sb = axp.tile([128, HT, S], BF16)  # 80KB/part
                # ---- A1: K/V projections ----
                NA = HT // 2 if FP8_PROJ else HT   # contraction tiles
                pm = DR if FP8_PROJ else None
                awp = actx.enter_context(tc.tile_pool(name=f"a_w{rep}", bufs=1))
                if FP8_PROJ:
                    wk_sb = awp.tile([128, NA, 2, KV * HD], FP8)     # 5KB/part
                    wv_sb = awp.tile([128, NA, 2, KV * HD], FP8)     # 5KB/part
                    wq_sb = awq.tile([128, NA, 2, HEADS * HD], FP8)  # 12.5KB/part
                    pieces = [0, 1, 2, 4, 6, 8, NA]
                else:
                    wk_sb = awp.tile([128, NA, KV * HD], BF16)   # 10KB/part
                    wv_sb = awp.tile([128, NA, KV * HD], BF16)   # 10KB/part
                    wq_sb = awq.tile([128, NA, HEADS * HD], BF16)  # 25KB/part
                    pieces = [0, 2, 4, 8, 12, 16, NA]

                def wslice(w_sb, a, d0, d1):
                    return w_sb[:, a, :, d0:d1] if FP8_PROJ else w_sb[:, a, d0:d1]

                def xslice(a, t0, t1):
                    return x_sb[:, a, :, t0:t1] if FP8_PROJ else x_sb[:, a, t0:t1]

                # interleave weight and x loads piece-by-piece: the shared DMA
                # path delivers exactly what the K chains need next
                for pc in range(len(pieces) - 1):
                    a0, a1 = pieces[pc], pieces[pc + 1]
                    nc.sync.dma_start(out=wk_sb[:, a0:a1], in_=wkT_t[:, a0:a1])
                    nc.sync.dma_start(out=x_sb[:, a0:a1], in_=xT_t[:, a0:a1])
                    if pc == 1:
                        nc.sync.dma_start(out=cos_sb, in_=cosT[:])
                        nc.sync.dma_start(out=sin_sb, in_=sinT[:])
                for pc in range(len(pieces) - 1):
                    a0, a1 = pieces[pc], pieces[pc + 1]
                    nc.sync.dma_start(out=wv_sb[:, a0:a1], in_=wvT_t[:, a0:a1])
                for pc in range(len(pieces) - 1):
                    a0, a1 = pieces[pc], pieces[pc + 1]
                    nc.sync.dma_start(out=wq_sb[:, a0:a1], in_=wqT_t[:, a0:a1])

                # K for ALL blocks, chains interleaved at x-piece granularity
                # (8 PSUM banks) so the PE chases the x DMA with no idle
                with ExitStack() as s0ctx:
                    sp0 = s0ctx.enter_context(
                        tc.tile_pool(name=f"a_s0{rep}", bufs=1, space="PSUM"))
                    pk0 = [[sp0.tile([128, BLK], F32, tag=f"pk{b}{kvh}",
                                     name=f"pk0_{b}_{kvh}") for kvh in range(KV)]
                           for b in range(NBLK)]
                    for pc in range(len(pieces) - 1):
                        for a in range(pieces[pc], pieces[pc + 1]):
                            for b in range(NBLK):
                                for kvh in range(KV):
                                    nc.tensor.matmul(pk0[b][kvh][:],
                                                     wslice(wk_sb, a, kvh * HD, (kvh + 1) * HD),
                                                     xslice(a, b * BLK, (b + 1) * BLK),
                                                     start=(a == 0), stop=(a == NA - 1),
                                                     perf_mode=pm)
                    for b in range(NBLK):
                        for kvh in range(KV):
                            rope(kT_sb[:, kvh, b * BLK:(b + 1) * BLK],
                                 pk0[b][kvh][:], b * BLK, BLK)

                # V projections, then Q projections
                app = actx.enter_context(
                    tc.tile_pool(name=f"a_ps{rep}", bufs=3, space="PSUM"))
                for blk in range(NBLK):
                    t0 = blk * BLK
                    for tt in range(BLK // 128):
                        pv = app.tile([128, KV * HD], F32, tag="pv")
                        for a in range(NA):
                            nc.tensor.matmul(pv[:], xslice(a, t0 + tt * 128, t0 + (tt + 1) * 128),
                                             wslice(wv_sb, a, 0, KV * HD),
                                             start=(a == 0), stop=(a == NA - 1),
                                             perf_mode=pm)
                        nc.scalar.copy(v_sb[:, blk * (BLK // 128) + tt, :], pv[:])

                # ---- A2: Q projection for block 0 only (blocks 1-3 are
                # interleaved into the BC chunk loop) ----
                for blk in range(1):
                    t0 = blk * BLK
                    for h in range(HEADS):
                        pq = app.tile([128, BLK], F32, tag="pp")
                        for a in range(NA):
                            nc.tensor.matmul(pq[:], wslice(wq_sb, a, h * HD, (h + 1) * HD),
                                             xslice(a, t0, t0 + BLK),
                                             start=(a == 0), stop=(a == NA - 1),
                                             perf_mode=pm)
                        rope(qT_sb[:, h, t0:t0 + BLK], pq[:], t0, BLK)

            # ---------- phase BC: attention + o_proj, per q-chunk ----------
            if a_only:
                # keep the projection results live (defeat dead-code elim):
                # dump kT/qT/v into disjoint y rows
                for h in range(HEADS):
                    nc.sync.dma_start(out=y[h * 128:(h + 1) * 128, 0:BLK],
                                      in_=qT_sb[:, h, 0:BLK])
                for kvh in range(KV):
                    nc.sync.dma_start(out=y[640 + kvh * 128:640 + (kvh + 1) * 128, 0:S],
                                      in_=kT_sb[:, kvh, :])
                nc.sync.dma_start(out=y[896:1024, 0:S], in_=v_sb[:, 0:8, :])
                nc.sync.dma_start(out=y[1024:1152, 0:S], in_=v_sb[:, 8:16, :])
                continue
            with ExitStack() as bctx:
                bwo = bctx.enter_context(tc.tile_pool(name=f"c_w{rep}", bufs=1))
                wo_sb = bwo.tile([128, HEADS, HID], BF16)   # 25KB/part
                nc.sync.dma_start(out=wo_sb[:, 0:2, :], in_=woT_t[:, 0:2, :])
                nc.sync.dma_start(out=wo_sb[:, 2:HEADS, :], in_=woT_t[:, 2:HEADS, :])
                bsp = bctx.enter_context(tc.tile_pool(name=f"b_s{rep}", bufs=2, space="PSUM"))
                bap = bctx.enter_context(tc.tile_pool(name=f"b_at{rep}", bufs=2, space="PSUM"))
                rpp = bctx.enter_context(tc.tile_pool(name=f"b_rp{rep}", bufs=2, space="PSUM"))
                bep = bctx.enter_context(tc.tile_pool(name=f"b_e{rep}", bufs=3))
                esp = bctx.enter_context(tc.tile_pool(name=f"b_es{rep}", bufs=2))
                bwp = bctx.enter_context(tc.tile_pool(name=f"b_w{rep}", bufs=2))
                atp = bctx.enter_context(tc.tile_pool(name=f"b_atc{rep}", bufs=2))
                ycp = bctx.enter_context(tc.tile_pool(name=f"c_y{rep}", bufs=1))
                # x block re-loads for the interleaved Q chains (idle DMA BW)
                bxq = bctx.enter_context(tc.tile_pool(name=f"b_xq{rep}", bufs=2))

                # o_proj for chunk cp, emitted as 5 slices of 4 (tt,n)-tiles
                # interleaved into the next chunk's head loop so the PE stays
                # fed while ACT works through that chunk's exps
                ystate = {}

                def oproj_slice(cp, at_prev, h):
                    if h == 0:
                        ystate["y"] = ycp.tile([128, 4, HID], BF16, tag="ych",
                                               name="y_ch")
                    y_ch = ystate["y"]
                    for k in range(4 * h, 4 * h + 4):
                        tt, n = divmod(k, NO)
                        py = rpp.tile([128, CH], F32, tag="rp", name="py")
                        for hh in range(HEADS):
                            nc.tensor.matmul(py[:], at_prev[:, hh, tt * 128:(tt + 1) * 128],
                                             wo_sb[:, hh, n * CH:(n + 1) * CH],
                                             start=(hh == 0), stop=(hh == HEADS - 1))
                        # PSUM->SBUF copies split across ACT and DVE: both are
                        # loaded in BC (exp vs esum/normalize), share the cost
                        if k % 2 == 0:
                            nc.scalar.copy(y_ch[:, tt, n * CH:(n + 1) * CH], py[:])
                        else:
                            nc.vector.tensor_copy(y_ch[:, tt, n * CH:(n + 1) * CH], py[:])
                        if n == NO - 1:
                            nc.sync.dma_start(out=y_r[:, cp, tt, :], in_=y_ch[:, tt, :])

                at_prev = None
                for c in range(NCH):
                    q0 = c * CH
                    ki_max = 4 * c + 3
                    if c < NCH - 1:
                        xq_sb = bxq.tile([128, HT, BLK], BF16, tag="xq")  # 20KB
                        nc.sync.dma_start(out=xq_sb,
                                          in_=xT_t[:, :, (c + 1) * BLK:(c + 2) * BLK])
                    at_ch = atp.tile([128, HEADS, CH], BF16, tag="atc")  # 5KB/part
                    for h in range(HEADS):
                        kvh = KVIDX[h]
                        pat = bap.tile([128, CH], F32, tag="pat")
                        esum = esp.tile([128, CH], F16, tag="esum")
                        # off-diagonal k-tiles, exp'd in 1024-wide pairs
                        for kp in range(2 * c):
                            ps = bsp.tile([128, 2 * CH], F32, tag="ps")
                            for j in range(2):
                                ki = 2 * kp + j
                                nc.tensor.matmul(ps[:, j * CH:(j + 1) * CH],
                                                 kT_sb[:, kvh, ki * 128:(ki + 1) * 128],
                                                 qT_sb[:, h, q0:q0 + CH],
                                                 start=True, stop=True)
                            et = bep.tile([128, 2 * CH], BF16, tag="et")
                            nc.scalar.activation(out=et[:], in_=ps[:],
                                                 func=mybir.ActivationFunctionType.Exp,
                                                 scale=SCALE)
                            for j in range(2):
                                ki = 2 * kp + j
                                nc.tensor.matmul(pat[:], v_sb[:, ki, kvh * HD:(kvh + 1) * HD],
                                                 et[:, j * CH:(j + 1) * CH],
                                                 start=(ki == 0), stop=False)
                            # denominator: pair-add on DVE, accumulate fp16
                            # (sums < 5e3 << fp16 max; 2-byte dtype = fast DVE)
                            if kp == 0:
                                nc.vector.tensor_add(esum[:], et[:, 0:CH], et[:, CH:2 * CH])
                            else:
                                tmp = esp.tile([128, CH], F16, tag="tmp")
                                nc.vector.tensor_add(tmp[:], et[:, 0:CH], et[:, CH:2 * CH])
                                nc.vector.tensor_add(esum[:], esum[:], tmp[:])
                        # diagonal band k-tiles (4c .. 4c+3)
                        for ki in range(4 * c, ki_max + 1):
                            ps = bsp.tile([128, 2 * CH], F32, tag="ps")
                            off = ki * 128 - q0
                            nc.tensor.matmul(ps[:, off:CH],
                                             kT_sb[:, kvh, ki * 128:(ki + 1) * 128],
                                             qT_sb[:, h, q0 + off:q0 + CH],
                                             start=True, stop=True)
                            et = bep.tile([128, 2 * CH], BF16, tag="et")
                            nc.scalar.activation(out=et[:, off:CH], in_=ps[:, off:CH],
                                                 func=mybir.ActivationFunctionType.Exp,
                                                 scale=SCALE)
                            nc.vector.tensor_mul(et[:, off:off + 128],
                                                 et[:, off:off + 128], mask_sb[:])
                            nc.tensor.matmul(pat[:, off:CH],
                                             v_sb[:, ki, kvh * HD:(kvh + 1) * HD],
                                             et[:, off:CH], start=(ki == 0),
                                             stop=(ki == ki_max))
                            if c == 0 and ki == 0:
                                nc.vector.tensor_copy(esum[:], et[:, 0:CH])
                            else:
                                nc.vector.tensor_add(esum[:, off:CH], esum[:, off:CH],
                                                     et[:, off:CH])
                        # o_proj slice of the previous chunk first: its PE work
                        # hides the DVE esum tail this head just queued
                        if c > 0:
                            oproj_slice(c - 1, at_prev, h)
                        # partition-reduce the fp16 esum with one ones-matmul
                        pR = rpp.tile([128, CH], F32, tag="rp")
                        nc.tensor.matmul(pR[:], ones_sb[:], esum[:], start=True, stop=True)
                        rec = bwp.tile([128, CH], F32, tag="rec")
                        nc.vector.reciprocal(rec[:], pR[:])
                        nc.vector.tensor_mul(at_ch[:, h, :], pat[:], rec[:])
                        # Q projection for (block c+1, head h): PE filler for
                        # this exp-bound stretch; consumed one chunk later
                        if c < NCH - 1:
                            t0 = (c + 1) * BLK
                            pq = rpp.tile([128, CH], F32, tag="rp", name="pq")
                            for a in range(HT):
                                nc.tensor.matmul(pq[:], wq_sb[:, a, h * HD:(h + 1) * HD],
                                                 xq_sb[:, a, :],
                                                 start=(a == 0), stop=(a == HT - 1))
                            rope(qT_sb[:, h, t0:t0 + BLK], pq[:], t0, BLK)
                    at_prev = at_ch
                for h in range(HEADS):
                    oproj_slice(NCH - 1, at_prev, h)

    _split_waits(nc)
    nc.finalize()
    return nc


def core_heads(g):
    """Query-head and kv-head global indices for core group g (= core % 4)."""
    qh = [4 * g, 4 * g + 1, 4 * g + 2, 4 * g + 3, 16 + g]
    kvh = [g, 4]
    return qh, kvh


def make_in_maps(hidden_states, position_ids, wq, wk, wv, wo, sub_w):
    hidden_states = np.asarray(hidden_states, dtype=np.float32)
    position_ids = np.asarray(position_ids)
    wq = np.asarray(wq, dtype=np.float32)
    wk = np.asarray(wk, dtype=np.float32)
    wv = np.asarray(wv, dtype=np.float32)
    wo = np.asarray(wo, dtype=np.float32)
    sub_w = np.asarray(sub_w, dtype=np.float32)

    wo_s = wo * sub_w[None, :]          # fold BitNetSubNorm gain into o_proj
    inv_freq = (1.0 / (THETA ** (np.arange(0, HD, 2, dtype=np.float32) / HD)))  # [64]
    mask01 = np.triu(np.ones((128, 128))).astype(ml_dtypes.bfloat16)

    bf = ml_dtypes.bfloat16
    if FP8_PROJ:
        f8 = mybir.dt.np(FP8)
        xsc, wsc, isc = X_SCALE, W_SCALE, 1.0 / (X_SCALE * W_SCALE)

        def cvt(m, s):
            return np.clip(m * s, -240.0, 240.0).astype(f8)
    else:
        xsc, wsc, isc = 1.0, 1.0, 1.0

        def cvt(m, s):
            return m.astype(bf)

    in_maps = []
    for c in range(NCORES):
        b, g = c // 4, c % 4
        qh, kvh = core_heads(g)
        qrows = np.concatenate([np.arange(h * HD, (h + 1) * HD) for h in qh])
        krows = np.concatenate([np.arange(k * HD, (k + 1) * HD) for k in kvh])

        pos = position_ids[b].astype(np.float32)                      # [S]
        ang = inv_freq[:, None] * pos[None, :]                        # [64, S]
        cosT = np.concatenate([np.cos(ang), np.cos(ang)], axis=0)     # [128, S]
        sinT = np.concatenate([-np.sin(ang), np.sin(ang)], axis=0)    # sign-folded

        in_maps.append({
            "xT": cvt(np.ascontiguousarray(hidden_states[b].T), xsc),  # [HID, S]
            "wqT": cvt(np.ascontiguousarray(wq[qrows].T), wsc),        # [HID, 640]
            "wkT": cvt(np.ascontiguousarray(wk[krows].T), wsc),        # [HID, 256]
            "wvT": cvt(np.ascontiguousarray(wv[krows].T), wsc),        # [HID, 256]
            "woT": np.ascontiguousarray(wo_s[:, qrows].T * isc).astype(bf),
            "cosT": np.ascontiguousarray(cosT * isc).astype(bf),
            "sinT": np.ascontiguousarray(sinT * isc).astype(bf),
            "mask": mask01,
            "ones": np.ones((128, 128), dtype=bf),
        })
    return in_maps


def kernel(hidden_states, position_ids, wq, wk, wv, wo, sub_w, _trace=False):
    if "nc" not in _CACHE:
        _CACHE["nc"] = build_nc()
    nc = _CACHE["nc"]
    in_maps = make_in_maps(hidden_states, position_ids, wq, wk, wv, wo, sub_w)
    res = run_bass_kernel_spmd(nc, in_maps, core_ids=list(range(NCORES)), trace=_trace)
    _CACHE["last_results"] = res
    out = np.zeros((B, S, HID), dtype=np.float32)
    for c in range(NCORES):
        out[c // 4] += res.results[c]["y"].astype(np.float32)
    return out
